# revision 9
# baseline (speedup 1.0000x reference)
"""Trainium2 Bass kernel for CrossAttentionAssociation.

Model: cross-attention (detections query tracks) + residual + LayerNorm,
then a pairwise association scorer:
  out[b,i,j] = sigmoid(w2 . relu(W1 (xn[b,i] * trk[b,j]) + b1) + b2)

Sharding (8 cores): core c handles batch b = c // 2 and detection rows
[256*(c%2), 256*(c%2)+256).  Tracks are replicated per batch.

Device-side structure (per core), v2:
- all matmuls float32r (full-rate fp32 streaming; bf16 MMs measured 2x
  SLOWER per column on this hw, so f32r everywhere)
- direct f32r DRAM->SBUF DMAs (no staging/copy pass)
- merged 4-head Q/K projections ([128,*] outputs instead of 8x[32,*])
- b_v folded host-side into the residual term (det + b_o + b_v @ w_o.T)
- scores for all 4 j-chunks of a head land in one [128,1024] 2-bank PSUM
  tile -> single exp instruction per head (amortizes ACT fixed cost)
- scorer a_i = w1T * xn_i built 32 detections at a time with ONE
  broadcast tensor_tensor per d-chunk (stride-0 APs)
- relu+bias split between Scalar (3 of 4) and Vector (1 of 4)
- w2 logit matmul emitted lagged 2 detections behind its relu so the PE
  queue never head-of-line blocks on ACT/DVE
- LN/attended for detection rows 128-255 emitted AFTER scorer group 0 so
  its cross-engine stalls hide under scorer matmuls
"""
import sys
import types

import numpy as np


def _install_ntff_hook():
    """Shim antenv.axon_hooks (absent on this image) so trace=True works."""
    if "antenv.axon_hooks" in sys.modules:
        return
    mod = types.ModuleType("antenv.axon_hooks")
    _hook = [None]
    mod.set_axon_ntff_profile_hook = lambda h: _hook.__setitem__(0, h)
    mod.get_axon_ntff_profile_hook = lambda: _hook[0]
    sys.modules["antenv.axon_hooks"] = mod
    try:
        from trn_agent_boot.trn_boot import _ntff_profile_via_ctypes
        mod.set_axon_ntff_profile_hook(
            _ntff_profile_via_ctypes("/opt/axon/libaxon_pjrt.so"))
    except Exception:
        pass


_install_ntff_hook()

import concourse.bacc as bacc  # noqa: E402
import concourse.mybir as mybir  # noqa: E402
import concourse.tile as tile  # noqa: E402
from concourse.bass_utils import run_bass_kernel_spmd  # noqa: E402

F32 = mybir.dt.float32
F32R = mybir.dt.float32r
AF = mybir.ActivationFunctionType
ALU = mybir.AluOpType

B, ND, NT, D = 4, 512, 512, 256
H, DK = 8, 32
DHID = 128
NDC = 256          # detections per core
LN_EPS = 1e-5
N_CORES = 8
GROUP = 32         # detections per logits/sigmoid group
NG = NDC // GROUP
V_RELU_MOD = 4     # i % 4 == 3 -> relu on Vector, else Scalar

_CACHE = {}


def _build():
    nc = bacc.Bacc("TRN2", target_bir_lowering=False, debug=False)

    def din(name, shape, dt=F32R):
        return nc.dram_tensor(name, shape, dt, kind="ExternalInput").ap()

    detT = din("detT", [D, NDC])          # det_chunk.T (f32r)
    det_bo = din("det_bo", [NDC, D], F32)  # det_chunk + b_o + b_v@w_o.T
    trkT = din("trkT", [D, NT])
    wqT = din("wqT", [D, D])
    wkT = din("wkT", [D, D])
    wvT = din("wvT", [D, D])
    woT = din("woT", [D, D])
    bq = din("bq", [D], F32)
    bk = din("bk", [D], F32)
    lng = din("lng", [D], F32)
    lnb = din("lnb", [D], F32)
    w1a = din("w1a", [128, DHID])         # w1T rows 0-127
    w1b = din("w1b", [128, DHID])         # w1T rows 128-255
    b1 = din("b1", [DHID], F32)
    w2s = din("w2s", [DHID, GROUP * GROUP])  # shifted stationary blocks
    b2b = din("b2b", [GROUP], F32)
    ident = din("ident", [128, 128], F32)
    out = nc.dram_tensor("out", [NDC, NT], F32, kind="ExternalOutput").ap()

    with tile.TileContext(nc) as tc:
        with (
            tc.tile_pool(name="persist", bufs=1) as pp,
            tc.tile_pool(name="stage", bufs=1) as stg,
        ):
            # ---- activation-table prefetch (exp set) under the DMA wait ----
            tiny = pp.tile([1, 4], F32)
            nc.vector.memset(tiny[:], 0.0)
            tiny2 = pp.tile([1, 4], F32)
            nc.scalar.activation(tiny2[:], tiny[:], AF.Exp)

            # ---- direct loads ----
            def load2(ap, f, dt=F32R, nm=None):
                nm = nm or ap.tensor.name
                ts = []
                for t in range(2):
                    s = pp.tile([128, f], dt, tag=f"ld_{nm}_{t}",
                                name=f"ld_{nm}_{t}")
                    nc.sync.dma_start(s[:], ap[t * 128:(t + 1) * 128, :])
                    ts.append(s)
                return ts

            trkT_r = load2(trkT, NT)
            detT_r = load2(detT, NDC)
            wqT_r = load2(wqT, D)
            wkT_r = load2(wkT, D)
            wvT_r = load2(wvT, D)
            woT_r = load2(woT, D)
            det_bo_t = load2(det_bo, D, F32)
            w1_t = []
            for nm, ap in (("w1a", w1a), ("w1b", w1b)):
                s = pp.tile([128, DHID], F32R, tag=nm, name=nm + "_t")
                nc.sync.dma_start(s[:], ap[:])
                w1_t.append(s)
            w2s_t = pp.tile([DHID, GROUP * GROUP], F32R)
            nc.sync.dma_start(w2s_t[:], w2s[:])
            idn = pp.tile([128, 128], F32)
            nc.sync.dma_start(idn[:], ident[:])

            def vec2(ap):
                nm = ap.tensor.name
                ts = []
                for t in range(2):
                    s = pp.tile([128, 1], F32, tag=f"v_{nm}_{t}",
                                name=f"v_{nm}_{t}")
                    nc.sync.dma_start(s[:, 0], ap[t * 128:(t + 1) * 128])
                    ts.append(s)
                return ts

            def vec4_64(ap):
                nm = ap.tensor.name
                ts = []
                for t in range(4):
                    s = pp.tile([64, 1], F32, tag=f"v4_{nm}_{t}",
                                name=f"v4_{nm}_{t}")
                    nc.sync.dma_start(s[:, 0], ap[t * 64:(t + 1) * 64])
                    ts.append(s)
                return ts

            bq_t = vec4_64(bq)
            bk_t = vec4_64(bk)
            lng_t = vec2(lng)
            lnb_t = vec2(lnb)
            b1_t = pp.tile([DHID, 1], F32)
            nc.sync.dma_start(b1_t[:, 0], b1[:])
            b2_t = pp.tile([GROUP, 1], F32)
            nc.sync.dma_start(b2_t[:, 0], b2b[:])
            eps_t = pp.tile([128, 1], F32)
            nc.vector.memset(eps_t[:], LN_EPS)

            # ---- merged K/Q projections (2 heads per [64,*] tile) ----
            km = []
            qm = []
            with tc.tile_pool(name="proj_ps", bufs=2, space="PSUM") as pps:
                for hb in range(4):
                    sl = slice(hb * 64, (hb + 1) * 64)
                    ps = pps.tile([64, NT], F32, tag="k")
                    for dc in range(2):
                        nc.tensor.matmul(ps[:], wkT_r[dc][:, sl],
                                         trkT_r[dc][:],
                                         start=(dc == 0), stop=(dc == 1))
                    k = pp.tile([64, NT], F32R, tag=f"km{hb}",
                                name=f"km{hb}")
                    nc.scalar.activation(k[:], ps[:], AF.Identity,
                                         bias=bk_t[hb][:])
                    km.append(k)

                    ps = pps.tile([64, NDC], F32, tag="q")
                    for dc in range(2):
                        nc.tensor.matmul(ps[:], wqT_r[dc][:, sl],
                                         detT_r[dc][:],
                                         start=(dc == 0), stop=(dc == 1))
                    q = pp.tile([64, NDC], F32R, tag=f"qm{hb}",
                                name=f"qm{hb}")
                    nc.scalar.activation(q[:], ps[:], AF.Identity,
                                         bias=bq_t[hb][:])
                    qm.append(q)

                # ---- V projection, token-major with ones/zero columns ----
                ones8 = pp.tile([128, H], F32)
                nc.vector.memset(ones8[:], 1.0)
                zero8 = pp.tile([128, H], F32)
                nc.vector.memset(zero8[:], 0.0)
                v_sb = []
                for jc in range(4):
                    ps = pps.tile([128, D], F32, tag="v")
                    for dc in range(2):
                        nc.tensor.matmul(
                            ps[:], trkT_r[dc][:, jc * 128:(jc + 1) * 128],
                            wvT_r[dc][:], start=(dc == 0), stop=(dc == 1))
                    v = pp.tile([128, H * 34], F32R, tag=f"vsb_{jc}",
                                name=f"vsb_{jc}")
                    vr = v.rearrange("p (h c) -> p h c", c=34)
                    nc.vector.tensor_copy(
                        vr[:, :, 0:32], ps.rearrange("p (h c) -> p h c", c=32))
                    nc.vector.tensor_copy(
                        vr[:, :, 32:33],
                        ones8.rearrange("p (h o) -> p h o", o=1))
                    nc.vector.tensor_copy(
                        vr[:, :, 33:34],
                        zero8.rearrange("p (h o) -> p h o", o=1))
                    v_sb.append(v)

            # ---- scores -> exp (one [128,1024] per head) -> ctx ----
            inv_sqrt_dk = 1.0 / np.sqrt(DK)
            ctxraw = [pp.tile([128, H * 34], F32, tag=f"ctxraw{ic}",
                              name=f"ctxraw{ic}") for ic in range(2)]
            with (
                tc.tile_pool(name="ctx_ps", bufs=1, space="PSUM") as cps,
                tc.tile_pool(name="eh_sb", bufs=3) as esb,
            ):
                psum_ctx = [cps.tile([128, H * 34], F32, tag=f"ctx{ic}",
                                     name=f"psum_ctx{ic}") for ic in range(2)]
                with tc.tile_pool(name="s_ps", bufs=2, space="PSUM") as sps:
                    pend = None  # (e, h) waiting for its ctx matmuls
                    for h in range(H):
                        hb, hsl = divmod(h, 2)
                        rs = slice(hsl * 32, (hsl + 1) * 32)
                        ps = sps.tile([128, 4 * NDC], F32, tag="s")
                        for jc in range(4):
                            nc.tensor.matmul(
                                ps[:, jc * NDC:(jc + 1) * NDC],
                                km[hb][rs, jc * 128:(jc + 1) * 128],
                                qm[hb][rs, :], start=True, stop=True)
                        e = esb.tile([128, 4 * NDC], F32R, tag="e")
                        nc.scalar.activation(e[:], ps[:], AF.Exp,
                                             scale=inv_sqrt_dk)
                        if pend is not None:
                            pe, ph_ = pend
                            for jc in range(4):
                                for ic in range(2):
                                    nc.tensor.matmul(
                                        psum_ctx[ic][:, ph_ * 34:
                                                     (ph_ + 1) * 34],
                                        pe[:, jc * NDC + ic * 128:
                                           jc * NDC + (ic + 1) * 128],
                                        v_sb[jc][:, ph_ * 34:(ph_ + 1) * 34],
                                        start=(jc == 0), stop=(jc == 3))
                        pend = (e, h)
                    pe, ph_ = pend
                    for jc in range(4):
                        for ic in range(2):
                            nc.tensor.matmul(
                                psum_ctx[ic][:, ph_ * 34:(ph_ + 1) * 34],
                                pe[:, jc * NDC + ic * 128:
                                   jc * NDC + (ic + 1) * 128],
                                v_sb[jc][:, ph_ * 34:(ph_ + 1) * 34],
                                start=(jc == 0), stop=(jc == 3))
                for ic in range(2):
                    nc.vector.tensor_copy(ctxraw[ic][:], psum_ctx[ic][:])

            if True:
                # ---- phase4: normalize ctx, attended, LN -> xnT ----
                xnT = [[pp.tile([128, 128], F32, tag=f"xnT{dc}_{ic}",
                                name=f"xnT{dc}_{ic}") for ic in range(2)]
                       for dc in range(2)]
                ctxT = [pp.tile([128, NDC], F32R, tag=f"ctxT{dc}",
                                name=f"ctxT{dc}") for dc in range(2)]

                def emit_phase4(ic, tps):
                    recip = stg.tile([128, H], F32, tag=f"recip{ic}",
                                     name=f"recip{ic}")
                    for h in range(H):
                        nc.vector.reciprocal(
                            recip[:, h:h + 1],
                            ctxraw[ic][:, h * 34 + 32:h * 34 + 33])
                    c = stg.tile([128, D], F32, tag=f"ctx_sb{ic}",
                                 name=f"ctx_sb{ic}")
                    for h in range(H):
                        nc.vector.tensor_scalar_mul(
                            c[:, h * DK:(h + 1) * DK],
                            ctxraw[ic][:, h * 34:h * 34 + 32],
                            recip[:, h:h + 1])
                    for dc in range(2):
                        pt = tps.tile([128, 128], F32, tag="tr")
                        nc.tensor.transpose(
                            pt[:], c[:, dc * 128:(dc + 1) * 128], idn[:])
                        nc.scalar.activation(
                            ctxT[dc][:, ic * 128:(ic + 1) * 128], pt[:],
                            AF.Copy)
                    ps = tps.tile([128, D], F32, tag="att")
                    for dc in range(2):
                        nc.tensor.matmul(
                            ps[:], ctxT[dc][:, ic * 128:(ic + 1) * 128],
                            woT_r[dc][:], start=(dc == 0), stop=(dc == 1))
                    x = stg.tile([128, D], F32, tag="x")
                    nc.vector.tensor_add(x[:], ps[:], det_bo_t[ic][:])
                    ssum = stg.tile([128, 1], F32, tag="ssum")
                    nc.vector.reduce_sum(ssum[:], x[:],
                                         axis=mybir.AxisListType.X)
                    mu = stg.tile([128, 1], F32, tag="mu")
                    nc.vector.tensor_scalar_mul(mu[:], ssum[:], 1.0 / D)
                    sq = stg.tile([128, D], F32, tag="sq")
                    ssq = stg.tile([128, 1], F32, tag="ssq")
                    nc.scalar.activation(sq[:], x[:], AF.Square,
                                         accum_out=ssq[:])
                    m2 = stg.tile([128, 1], F32, tag="m2")
                    nc.vector.tensor_scalar_mul(m2[:], ssq[:], 1.0 / D)
                    mu2 = stg.tile([128, 1], F32, tag="mu2")
                    nc.vector.tensor_mul(mu2[:], mu[:], mu[:])
                    var = stg.tile([128, 1], F32, tag="var")
                    nc.vector.tensor_sub(var[:], m2[:], mu2[:])
                    sd = stg.tile([128, 1], F32, tag="sd")
                    nc.scalar.activation(sd[:], var[:], AF.Sqrt,
                                         bias=eps_t[:])
                    rstd = stg.tile([128, 1], F32, tag="rstd")
                    nc.vector.reciprocal(rstd[:], sd[:])
                    y = stg.tile([128, D], F32, tag="y")
                    nc.vector.tensor_scalar(
                        y[:], x[:], mu[:], rstd[:],
                        op0=ALU.subtract, op1=ALU.mult)
                    for dc in range(2):
                        pt = tps.tile([128, 128], F32, tag="tr")
                        nc.tensor.transpose(
                            pt[:], y[:, dc * 128:(dc + 1) * 128], idn[:])
                        nc.vector.tensor_scalar(
                            xnT[dc][ic][:], pt[:],
                            lng_t[dc][:], lnb_t[dc][:],
                            op0=ALU.mult, op1=ALU.add)

                # ---- association scorer ----
                with (
                    tc.tile_pool(name="a_sb", bufs=2) as asb,
                    tc.tile_pool(name="r_sb", bufs=4) as rsb,
                    tc.tile_pool(name="tr_ps", bufs=1, space="PSUM") as tps,
                    tc.tile_pool(name="h_ps", bufs=3, space="PSUM") as hps,
                    tc.tile_pool(name="l_ps", bufs=2, space="PSUM") as lqs,
                    tc.tile_pool(name="sig_sb", bufs=2) as ssb,
                ):
                    emit_phase4(0, tps)

                    ablocks = {}

                    def emit_ablk(g):
                        ic, c0 = divmod(g, 4)
                        c0 *= GROUP
                        blks = []
                        for dc in range(2):
                            a = asb.tile([128, GROUP * DHID], F32R,
                                         tag=f"a{dc}", name=f"ablk{dc}_{g}")
                            nc.vector.tensor_tensor(
                                a.rearrange("p (i h) -> p i h", h=DHID),
                                xnT[dc][ic][:, c0:c0 + GROUP]
                                .to_broadcast([128, GROUP, DHID]),
                                w1_t[dc][:].unsqueeze(1)
                                .to_broadcast([128, GROUP, DHID]),
                                op=ALU.mult)
                            blks.append(a)
                        ablocks[g] = blks

                    emit_ablk(0)

                    for g in range(NG):
                        a0, a1 = ablocks.pop(g)
                        psum_l = lqs.tile([GROUP, NT], F32, tag="l")
                        w2q = []  # (r, rt) lagged w2 matmuls
                        for r in range(GROUP):
                            ph = hps.tile([128, NT], F32, tag="h")
                            nc.tensor.matmul(
                                ph[:], a0[:, r * DHID:(r + 1) * DHID],
                                trkT_r[0][:], start=True, stop=False)
                            if len(w2q) > 2:
                                rq, rtq = w2q.pop(0)
                                nc.tensor.matmul(
                                    psum_l[:],
                                    w2s_t[:, rq * GROUP:(rq + 1) * GROUP],
                                    rtq[:], start=(rq == 0), stop=False)
                            nc.tensor.matmul(
                                ph[:], a1[:, r * DHID:(r + 1) * DHID],
                                trkT_r[1][:], start=False, stop=True)
                            rt = rsb.tile([128, NT], F32R, tag="r")
                            if r % V_RELU_MOD == V_RELU_MOD - 1:
                                nc.vector.tensor_scalar(
                                    rt[:], ph[:], b1_t[:], 0.0,
                                    op0=ALU.add, op1=ALU.max)
                            else:
                                nc.scalar.activation(rt[:], ph[:], AF.Relu,
                                                     bias=b1_t[:])
                            w2q.append((r, rt))
                            if r == 15 and g + 1 < NG:
                                emit_ablk(g + 1)
                        for rq, rtq in w2q:
                            nc.tensor.matmul(
                                psum_l[:],
                                w2s_t[:, rq * GROUP:(rq + 1) * GROUP],
                                rtq[:], start=(rq == 0),
                                stop=(rq == GROUP - 1))
                        sg = ssb.tile([GROUP, NT], F32, tag="sig")
                        nc.scalar.activation(sg[:], psum_l[:], AF.Sigmoid,
                                             bias=b2_t[:])
                        nc.sync.dma_start(
                            out[g * GROUP:(g + 1) * GROUP, :], sg[:])
                        if g == 0:
                            emit_phase4(1, tps)

    nc.compile()
    return nc


def _host_prep(inputs):
    """Build the 8 per-core input maps from full inputs (numpy, cheap)."""
    det = np.ascontiguousarray(inputs["detections"], np.float32)
    trk = np.ascontiguousarray(inputs["tracks"], np.float32)
    f32 = lambda x: np.ascontiguousarray(np.asarray(x), np.float32)
    w_q, b_q = f32(inputs["w_q"]), f32(inputs["b_q"])
    w_k, b_k = f32(inputs["w_k"]), f32(inputs["b_k"])
    w_v, b_v = f32(inputs["w_v"]), f32(inputs["b_v"])
    w_o, b_o = f32(inputs["w_o"]), f32(inputs["b_o"])
    ln_g, ln_b = f32(inputs["ln_g"]), f32(inputs["ln_b"])
    w1, b1 = f32(inputs["w1"]), f32(inputs["b1"])
    w2, b2 = f32(inputs["w2"]), f32(inputs["b2"])

    w2s = np.zeros((DHID, GROUP * GROUP), np.float32)
    for r in range(GROUP):
        w2s[:, r * GROUP + r] = w2[0]
    bo_eff = b_o + b_v @ w_o.T   # b_v folded through the output projection
    w1T = np.ascontiguousarray(w1.T)
    shared = {
        "wqT": np.ascontiguousarray(w_q.T), "wkT": np.ascontiguousarray(w_k.T),
        "wvT": np.ascontiguousarray(w_v.T), "woT": np.ascontiguousarray(w_o.T),
        "bq": b_q, "bk": b_k,
        "lng": ln_g, "lnb": ln_b,
        "w1a": np.ascontiguousarray(w1T[0:128]),
        "w1b": np.ascontiguousarray(w1T[128:256]),
        "b1": b1,
        "w2s": w2s, "b2b": np.full(GROUP, b2[0], np.float32),
        "ident": np.eye(128, dtype=np.float32),
    }
    in_maps = []
    for c in range(N_CORES):
        b, half = divmod(c, 2)
        dchunk = det[b, half * NDC:(half + 1) * NDC, :]
        m = dict(shared)
        m["detT"] = np.ascontiguousarray(dchunk.T)
        m["det_bo"] = np.ascontiguousarray(dchunk + bo_eff[None, :])
        m["trkT"] = np.ascontiguousarray(trk[b].T)
        in_maps.append(m)
    return in_maps


def _get_nc():
    if "nc" not in _CACHE:
        _CACHE["nc"] = _build()
    return _CACHE["nc"]


def run(inputs, trace=False):
    nc = _get_nc()
    in_maps = _host_prep(inputs)
    res = run_bass_kernel_spmd(nc, in_maps, core_ids=list(range(N_CORES)),
                               trace=trace)
    full = np.empty((B, ND, NT), np.float32)
    for c in range(N_CORES):
        b, half = divmod(c, 2)
        full[b, half * NDC:(half + 1) * NDC, :] = res.results[c]["out"]
    return full, res


def kernel(**inputs):
    return run(inputs, trace=False)[0]


# revision 17
# speedup vs baseline: 1.2161x; 1.2161x over previous
"""Trainium2 Bass kernel for CrossAttentionAssociation.

Model: cross-attention (detections query tracks) + residual + LayerNorm,
then a pairwise association scorer:
  out[b,i,j] = sigmoid(w2 . relu(W1 (xn[b,i] * trk[b,j]) + b1) + b2)

Sharding (8 cores): core c handles batch b = c // 2 and detection rows
[256*(c%2), 256*(c%2)+256).  Tracks are replicated per batch.

Device-side structure (per core), v2:
- all matmuls float32r (full-rate fp32 streaming; bf16 MMs measured 2x
  SLOWER per column on this hw, so f32r everywhere)
- direct f32r DRAM->SBUF DMAs (no staging/copy pass)
- merged 4-head Q/K projections ([128,*] outputs instead of 8x[32,*])
- b_v folded host-side into the residual term (det + b_o + b_v @ w_o.T)
- scores for all 4 j-chunks of a head land in one [128,1024] 2-bank PSUM
  tile -> single exp instruction per head (amortizes ACT fixed cost)
- scorer a_i = w1T * xn_i built 32 detections at a time with ONE
  broadcast tensor_tensor per d-chunk (stride-0 APs)
- relu+bias split between Scalar (3 of 4) and Vector (1 of 4)
- w2 logit matmul emitted lagged 2 detections behind its relu so the PE
  queue never head-of-line blocks on ACT/DVE
- LN/attended for detection rows 128-255 emitted AFTER scorer group 0 so
  its cross-engine stalls hide under scorer matmuls
"""
import sys
import types

import numpy as np


def _install_ntff_hook():
    """Shim antenv.axon_hooks (absent on this image) so trace=True works."""
    if "antenv.axon_hooks" in sys.modules:
        return
    mod = types.ModuleType("antenv.axon_hooks")
    _hook = [None]
    mod.set_axon_ntff_profile_hook = lambda h: _hook.__setitem__(0, h)
    mod.get_axon_ntff_profile_hook = lambda: _hook[0]
    sys.modules["antenv.axon_hooks"] = mod
    try:
        from trn_agent_boot.trn_boot import _ntff_profile_via_ctypes
        mod.set_axon_ntff_profile_hook(
            _ntff_profile_via_ctypes("/opt/axon/libaxon_pjrt.so"))
    except Exception:
        pass


_install_ntff_hook()

import concourse.bacc as bacc  # noqa: E402
import concourse.mybir as mybir  # noqa: E402
import concourse.tile as tile  # noqa: E402
from concourse.bass_utils import run_bass_kernel_spmd  # noqa: E402

F32 = mybir.dt.float32
F32R = mybir.dt.float32r
AF = mybir.ActivationFunctionType
ALU = mybir.AluOpType

B, ND, NT, D = 4, 512, 512, 256
H, DK = 8, 32
DHID = 128
NDC = 256          # detections per core
LN_EPS = 1e-5
N_CORES = 8
GROUP = 32         # detections per logits/sigmoid group
NG = NDC // GROUP
V_RELU_MOD = 4     # i % 4 == 3 -> relu on Vector, else Scalar

_CACHE = {}


def _build():
    nc = bacc.Bacc("TRN2", target_bir_lowering=False, debug=False)

    def din(name, shape, dt=F32R):
        return nc.dram_tensor(name, shape, dt, kind="ExternalInput").ap()

    # column-packed load bundles (see _host_prep for layouts)
    bigR = din("bigR", [128, 4864])        # f32r weights/activations bundle
    bigF = din("bigF", [128, 640], F32)    # det_bo chunks + identity
    vecs = din("vecs", [128, 14], F32)     # bias/scale vectors
    out = nc.dram_tensor("out", [NDC, NT], F32, kind="ExternalOutput").ap()

    with tile.TileContext(nc) as tc:
        with (
            tc.tile_pool(name="persist", bufs=1) as pp,
            tc.tile_pool(name="stage", bufs=1) as stg,
        ):
            # ---- activation-table prefetch (exp set) under the DMA wait ----
            tiny = pp.tile([1, 4], F32)
            nc.vector.memset(tiny[:], 0.0)
            tiny2 = pp.tile([1, 4], F32)
            nc.scalar.activation(tiny2[:], tiny[:], AF.Exp)

            # ---- PE warm-up: un-throttle HAM during the DMA wait ----
            wz = pp.tile([128, 512], mybir.dt.bfloat16)
            nc.vector.memset(wz[:], 0.0)
            with tc.tile_pool(name="wup_ps", bufs=1, space="PSUM") as wps:
                wp = wps.tile([128, 512], F32)
                for _ in range(14):
                    nc.tensor.matmul(wp[:], wz[:, 0:128], wz[:],
                                     start=True, stop=True)

            # ---- batched loads (3 + 1 + 1 DMAs) ----
            bigR_t = pp.tile([128, 4864], F32R, name="bigR_t")
            nc.sync.dma_start(bigR_t[:, 0:2048], bigR[:, 0:2048])
            vecs_t = pp.tile([128, 14], F32, name="vecs_t")
            nc.sync.dma_start(vecs_t[:], vecs[:])
            nc.sync.dma_start(bigR_t[:, 2048:3072], bigR[:, 2048:3072])
            bigF_t = pp.tile([128, 640], F32, name="bigF_t")
            nc.sync.dma_start(bigF_t[:], bigF[:])
            nc.sync.dma_start(bigR_t[:, 3072:4864], bigR[:, 3072:4864])

            trkT_r = [bigR_t[:, 0:512], bigR_t[:, 512:1024]]
            wkT_r = [bigR_t[:, 1024:1280], bigR_t[:, 1280:1536]]
            wvT_r = [bigR_t[:, 1536:1792], bigR_t[:, 1792:2048]]
            detT_r = [bigR_t[:, 2048:2304], bigR_t[:, 2304:2560]]
            wqT_r = [bigR_t[:, 2560:2816], bigR_t[:, 2816:3072]]
            woT_r = [bigR_t[:, 3072:3328], bigR_t[:, 3328:3584]]
            w1_t = [bigR_t[:, 3584:3712], bigR_t[:, 3712:3840]]
            w2s_t = bigR_t[:, 3840:4864]
            det_bo_t = [bigF_t[:, 0:256], bigF_t[:, 256:512]]
            idn = bigF_t[:, 512:640]
            bq_t = [vecs_t[0:64, hb:hb + 1] for hb in range(4)]
            bk_t = [vecs_t[0:64, 4 + hb:5 + hb] for hb in range(4)]
            lng_t = [vecs_t[:, 8 + t:9 + t] for t in range(2)]
            lnb_t = [vecs_t[:, 10 + t:11 + t] for t in range(2)]
            b1_t = vecs_t[:, 12:13]
            b2_t = vecs_t[0:32, 13:14]
            eps_t = pp.tile([128, 1], F32)
            nc.vector.memset(eps_t[:], LN_EPS)

            # ---- merged K/Q projections (2 heads per [64,*] tile) ----
            km = []
            qm = []
            with tc.tile_pool(name="proj_ps", bufs=2, space="PSUM") as pps:
                for hb in range(4):
                    sl = slice(hb * 64, (hb + 1) * 64)
                    ps = pps.tile([64, NT], F32, tag="k")
                    for dc in range(2):
                        nc.tensor.matmul(ps[:], wkT_r[dc][:, sl],
                                         trkT_r[dc][:],
                                         start=(dc == 0), stop=(dc == 1))
                    k = pp.tile([64, NT], F32R, tag=f"km{hb}",
                                name=f"km{hb}")
                    nc.scalar.activation(k[:], ps[:], AF.Identity,
                                         bias=bk_t[hb][:])
                    km.append(k)

                    ps = pps.tile([64, NDC], F32, tag="q")
                    for dc in range(2):
                        nc.tensor.matmul(ps[:], wqT_r[dc][:, sl],
                                         detT_r[dc][:],
                                         start=(dc == 0), stop=(dc == 1))
                    q = pp.tile([64, NDC], F32R, tag=f"qm{hb}",
                                name=f"qm{hb}")
                    nc.scalar.activation(q[:], ps[:], AF.Identity,
                                         bias=bq_t[hb][:])
                    qm.append(q)

                # ---- V projection, token-major with ones/zero columns ----
                ones8 = pp.tile([128, H], F32)
                nc.vector.memset(ones8[:], 1.0)
                zero8 = pp.tile([128, H], F32)
                nc.vector.memset(zero8[:], 0.0)
                v_sb = []
                for jc in range(4):
                    ps = pps.tile([128, D], F32, tag="v")
                    for dc in range(2):
                        nc.tensor.matmul(
                            ps[:], trkT_r[dc][:, jc * 128:(jc + 1) * 128],
                            wvT_r[dc][:], start=(dc == 0), stop=(dc == 1))
                    v = pp.tile([128, H * 34], F32R, tag=f"vsb_{jc}",
                                name=f"vsb_{jc}")
                    vr = v.rearrange("p (h c) -> p h c", c=34)
                    nc.vector.tensor_copy(
                        vr[:, :, 0:32], ps.rearrange("p (h c) -> p h c", c=32))
                    nc.vector.tensor_copy(
                        vr[:, :, 32:33],
                        ones8.rearrange("p (h o) -> p h o", o=1))
                    nc.vector.tensor_copy(
                        vr[:, :, 33:34],
                        zero8.rearrange("p (h o) -> p h o", o=1))
                    v_sb.append(v)

            # ---- scores -> exp (one [128,1024] per head) -> ctx ----
            inv_sqrt_dk = 1.0 / np.sqrt(DK)
            ctxraw = [pp.tile([128, H * 34], F32, tag=f"ctxraw{ic}",
                              name=f"ctxraw{ic}") for ic in range(2)]
            with (
                tc.tile_pool(name="ctx_ps", bufs=1, space="PSUM") as cps,
                tc.tile_pool(name="eh_sb", bufs=3) as esb,
            ):
                psum_ctx = [cps.tile([128, H * 34], F32, tag=f"ctx{ic}",
                                     name=f"psum_ctx{ic}") for ic in range(2)]
                with tc.tile_pool(name="s_ps", bufs=2, space="PSUM") as sps:
                    pend = None  # (e, h) waiting for its ctx matmuls
                    for h in range(H):
                        hb, hsl = divmod(h, 2)
                        rs = slice(hsl * 32, (hsl + 1) * 32)
                        ps = sps.tile([128, 4 * NDC], F32, tag="s")
                        for jc in range(4):
                            nc.tensor.matmul(
                                ps[:, jc * NDC:(jc + 1) * NDC],
                                km[hb][rs, jc * 128:(jc + 1) * 128],
                                qm[hb][rs, :], start=True, stop=True)
                        e = esb.tile([128, 4 * NDC], F32R, tag="e")
                        nc.scalar.activation(e[:], ps[:], AF.Exp,
                                             scale=inv_sqrt_dk)
                        if pend is not None:
                            pe, ph_ = pend
                            for jc in range(4):
                                for ic in range(2):
                                    nc.tensor.matmul(
                                        psum_ctx[ic][:, ph_ * 34:
                                                     (ph_ + 1) * 34],
                                        pe[:, jc * NDC + ic * 128:
                                           jc * NDC + (ic + 1) * 128],
                                        v_sb[jc][:, ph_ * 34:(ph_ + 1) * 34],
                                        start=(jc == 0), stop=(jc == 3))
                        pend = (e, h)
                    pe, ph_ = pend
                    for jc in range(4):
                        for ic in range(2):
                            nc.tensor.matmul(
                                psum_ctx[ic][:, ph_ * 34:(ph_ + 1) * 34],
                                pe[:, jc * NDC + ic * 128:
                                   jc * NDC + (ic + 1) * 128],
                                v_sb[jc][:, ph_ * 34:(ph_ + 1) * 34],
                                start=(jc == 0), stop=(jc == 3))
                for ic in range(2):
                    nc.vector.tensor_copy(ctxraw[ic][:], psum_ctx[ic][:])

            if True:
                # ---- phase4: normalize ctx, attended, LN -> xnT ----
                xnT = [[pp.tile([128, 128], F32, tag=f"xnT{dc}_{ic}",
                                name=f"xnT{dc}_{ic}") for ic in range(2)]
                       for dc in range(2)]
                ctxT = [pp.tile([128, NDC], F32R, tag=f"ctxT{dc}",
                                name=f"ctxT{dc}") for dc in range(2)]

                def emit_phase4(ic, tps):
                    recip = stg.tile([128, H], F32, tag=f"recip{ic}",
                                     name=f"recip{ic}")
                    for h in range(H):
                        nc.vector.reciprocal(
                            recip[:, h:h + 1],
                            ctxraw[ic][:, h * 34 + 32:h * 34 + 33])
                    c = stg.tile([128, D], F32, tag=f"ctx_sb{ic}",
                                 name=f"ctx_sb{ic}")
                    for h in range(H):
                        nc.vector.tensor_scalar_mul(
                            c[:, h * DK:(h + 1) * DK],
                            ctxraw[ic][:, h * 34:h * 34 + 32],
                            recip[:, h:h + 1])
                    for dc in range(2):
                        pt = tps.tile([128, 128], F32, tag="tr")
                        nc.tensor.transpose(
                            pt[:], c[:, dc * 128:(dc + 1) * 128], idn[:])
                        nc.scalar.activation(
                            ctxT[dc][:, ic * 128:(ic + 1) * 128], pt[:],
                            AF.Copy)
                    ps = tps.tile([128, D], F32, tag="att")
                    for dc in range(2):
                        nc.tensor.matmul(
                            ps[:], ctxT[dc][:, ic * 128:(ic + 1) * 128],
                            woT_r[dc][:], start=(dc == 0), stop=(dc == 1))
                    x = stg.tile([128, D], F32, tag="x")
                    nc.vector.tensor_add(x[:], ps[:], det_bo_t[ic][:])
                    ssum = stg.tile([128, 1], F32, tag="ssum")
                    nc.vector.reduce_sum(ssum[:], x[:],
                                         axis=mybir.AxisListType.X)
                    mu = stg.tile([128, 1], F32, tag="mu")
                    nc.vector.tensor_scalar_mul(mu[:], ssum[:], 1.0 / D)
                    sq = stg.tile([128, D], F32, tag="sq")
                    ssq = stg.tile([128, 1], F32, tag="ssq")
                    nc.scalar.activation(sq[:], x[:], AF.Square,
                                         accum_out=ssq[:])
                    m2 = stg.tile([128, 1], F32, tag="m2")
                    nc.vector.tensor_scalar_mul(m2[:], ssq[:], 1.0 / D)
                    mu2 = stg.tile([128, 1], F32, tag="mu2")
                    nc.vector.tensor_mul(mu2[:], mu[:], mu[:])
                    var = stg.tile([128, 1], F32, tag="var")
                    nc.vector.tensor_sub(var[:], m2[:], mu2[:])
                    sd = stg.tile([128, 1], F32, tag="sd")
                    nc.scalar.activation(sd[:], var[:], AF.Sqrt,
                                         bias=eps_t[:])
                    rstd = stg.tile([128, 1], F32, tag="rstd")
                    nc.vector.reciprocal(rstd[:], sd[:])
                    y = stg.tile([128, D], F32, tag="y")
                    nc.vector.tensor_scalar(
                        y[:], x[:], mu[:], rstd[:],
                        op0=ALU.subtract, op1=ALU.mult)
                    for dc in range(2):
                        pt = tps.tile([128, 128], F32, tag="tr")
                        nc.tensor.transpose(
                            pt[:], y[:, dc * 128:(dc + 1) * 128], idn[:])
                        nc.vector.tensor_scalar(
                            xnT[dc][ic][:], pt[:],
                            lng_t[dc][:], lnb_t[dc][:],
                            op0=ALU.mult, op1=ALU.add)

                # ---- association scorer ----
                with (
                    tc.tile_pool(name="a_sb", bufs=2) as asb,
                    tc.tile_pool(name="r_sb", bufs=6) as rsb,
                    tc.tile_pool(name="tr_ps", bufs=1, space="PSUM") as tps,
                    tc.tile_pool(name="h_ps", bufs=4, space="PSUM") as hps,
                    tc.tile_pool(name="l_ps", bufs=2, space="PSUM") as lqs,
                    tc.tile_pool(name="sig_sb", bufs=2) as ssb,
                ):
                    emit_phase4(0, tps)

                    ablocks = {}
                    HG = GROUP // 2

                    def alloc_ablk(g):
                        blks = [asb.tile([128, GROUP * DHID], F32R,
                                         tag=f"a{dc}", name=f"ablk{dc}_{g}")
                                for dc in range(2)]
                        ablocks[g] = blks

                    def emit_ablk_half(g, dc, hf):
                        ic, c0 = divmod(g, 4)
                        c0 = c0 * GROUP + hf * HG
                        a = ablocks[g][dc]
                        nc.vector.tensor_tensor(
                            a[:, hf * HG * DHID:(hf + 1) * HG * DHID]
                            .rearrange("p (i h) -> p i h", h=DHID),
                            xnT[dc][ic][:, c0:c0 + HG]
                            .to_broadcast([128, HG, DHID]),
                            w1_t[dc][:].unsqueeze(1)
                            .to_broadcast([128, HG, DHID]),
                            op=ALU.mult)

                    alloc_ablk(0)
                    for dc in range(2):
                        for hf in range(2):
                            emit_ablk_half(0, dc, hf)

                    psum_ls = {}
                    sig_done = 0
                    w2q = []  # (i, rt) lagged w2 matmuls across groups

                    def emit_w2(iq, rtq):
                        gq, rq = divmod(iq, GROUP)
                        nc.tensor.matmul(
                            psum_ls[gq][:],
                            w2s_t[:, rq * GROUP:(rq + 1) * GROUP],
                            rtq[:], start=(rq == 0), stop=(rq == GROUP - 1))
                        if rq == GROUP - 1:
                            sg = ssb.tile([GROUP, NT], F32, tag="sig")
                            nc.scalar.activation(sg[:], psum_ls[gq][:],
                                                 AF.Sigmoid, bias=b2_t[:])
                            nc.sync.dma_start(
                                out[gq * GROUP:(gq + 1) * GROUP, :], sg[:])
                            del psum_ls[gq]

                    for i in range(NDC):
                        g, r = divmod(i, GROUP)
                        if r == 0:
                            psum_ls[g] = lqs.tile([GROUP, NT], F32, tag="l", name=f"psum_l{g}")
                            a0, a1 = ablocks[g]
                        ph = hps.tile([128, NT], F32, tag="h")
                        nc.tensor.matmul(
                            ph[:], a0[:, r * DHID:(r + 1) * DHID],
                            trkT_r[0][:], start=True, stop=False)
                        if len(w2q) > 3:
                            emit_w2(*w2q.pop(0))
                        nc.tensor.matmul(
                            ph[:], a1[:, r * DHID:(r + 1) * DHID],
                            trkT_r[1][:], start=False, stop=True)
                        rt = rsb.tile([128, NT], F32R, tag="r")
                        if r % V_RELU_MOD == 1:
                            nc.vector.tensor_scalar(
                                rt[:], ph[:], b1_t[:], 0.0,
                                op0=ALU.add, op1=ALU.max)
                        else:
                            nc.scalar.activation(rt[:], ph[:], AF.Relu,
                                                 bias=b1_t[:])
                        w2q.append((i, rt))
                        if g + 1 < NG:
                            if r == 2:
                                alloc_ablk(g + 1)
                                emit_ablk_half(g + 1, 0, 0)
                            elif r == 9:
                                emit_ablk_half(g + 1, 1, 0)
                            elif r == 17:
                                emit_ablk_half(g + 1, 0, 1)
                            elif r == 24:
                                emit_ablk_half(g + 1, 1, 1)
                        if i == GROUP + 2:
                            emit_phase4(1, tps)
                        if r == GROUP - 1:
                            ablocks.pop(g)
                    while w2q:
                        emit_w2(*w2q.pop(0))

    nc.compile()
    return nc


def _host_prep(inputs):
    """Build the 8 per-core input maps from full inputs (numpy, cheap)."""
    det = np.ascontiguousarray(inputs["detections"], np.float32)
    trk = np.ascontiguousarray(inputs["tracks"], np.float32)
    f32 = lambda x: np.ascontiguousarray(np.asarray(x), np.float32)
    w_q, b_q = f32(inputs["w_q"]), f32(inputs["b_q"])
    w_k, b_k = f32(inputs["w_k"]), f32(inputs["b_k"])
    w_v, b_v = f32(inputs["w_v"]), f32(inputs["b_v"])
    w_o, b_o = f32(inputs["w_o"]), f32(inputs["b_o"])
    ln_g, ln_b = f32(inputs["ln_g"]), f32(inputs["ln_b"])
    w1, b1 = f32(inputs["w1"]), f32(inputs["b1"])
    w2, b2 = f32(inputs["w2"]), f32(inputs["b2"])

    w2s = np.zeros((DHID, GROUP * GROUP), np.float32)
    for r in range(GROUP):
        w2s[:, r * GROUP + r] = w2[0]
    bo_eff = b_o + b_v @ w_o.T   # b_v folded through the output projection
    w1T = np.ascontiguousarray(w1.T)

    def chunks(m):  # [256, F] -> ([128, F], [128, F])
        return m[0:128], m[128:256]

    wkT = w_k.T
    wvT = w_v.T
    wqT = w_q.T
    woT = w_o.T
    # bigR column layout (f32r):
    #  trkT0 trkT1 | wkT0 wkT1 wvT0 wvT1 | detT0 detT1 wqT0 wqT1 |
    #  woT0 woT1 w1a w1b w2s
    w_part = np.concatenate(
        [*chunks(wkT), *chunks(wvT)], axis=1)            # [128, 1024]
    tail = np.concatenate(
        [*chunks(woT), w1T[0:128], w1T[128:256], w2s], axis=1)  # [128, 1792]
    q_part = np.concatenate([*chunks(wqT)], axis=1)      # [128, 512]

    vecs = np.zeros((128, 14), np.float32)
    for hb in range(4):
        vecs[0:64, hb] = b_q[hb * 64:(hb + 1) * 64]
        vecs[0:64, 4 + hb] = b_k[hb * 64:(hb + 1) * 64]
    for t in range(2):
        vecs[:, 8 + t] = ln_g[t * 128:(t + 1) * 128]
        vecs[:, 10 + t] = ln_b[t * 128:(t + 1) * 128]
    vecs[:, 12] = b1
    vecs[0:32, 13] = b2[0]

    bigF_tail = np.eye(128, dtype=np.float32)
    in_maps = []
    for c in range(N_CORES):
        b, half = divmod(c, 2)
        dchunk = det[b, half * NDC:(half + 1) * NDC, :]
        detT = dchunk.T
        trkT = trk[b].T
        bigR = np.concatenate(
            [trkT[0:128], trkT[128:256], w_part,
             detT[0:128], detT[128:256], q_part, tail], axis=1)
        det_bo = dchunk + bo_eff[None, :]
        # det_bo chunks are [128, 256] row-blocks of the [256, 256] matrix
        bigF = np.concatenate(
            [det_bo[0:128], det_bo[128:256], bigF_tail], axis=1)
        m = {
            "bigR": np.ascontiguousarray(bigR),
            "bigF": np.ascontiguousarray(bigF),
            "vecs": vecs,
        }
        in_maps.append(m)
    return in_maps


def _get_nc():
    if "nc" not in _CACHE:
        _CACHE["nc"] = _build()
    return _CACHE["nc"]


def run(inputs, trace=False):
    nc = _get_nc()
    in_maps = _host_prep(inputs)
    res = run_bass_kernel_spmd(nc, in_maps, core_ids=list(range(N_CORES)),
                               trace=trace)
    full = np.empty((B, ND, NT), np.float32)
    for c in range(N_CORES):
        b, half = divmod(c, 2)
        full[b, half * NDC:(half + 1) * NDC, :] = res.results[c]["out"]
    return full, res


def kernel(**inputs):
    return run(inputs, trace=False)[0]
